# revision 41
# baseline (speedup 1.0000x reference)
"""Trainium2 Bass kernel for nn_CANDY_41077067219071.

Computation (per channel c of 64, H = I = 1024):
    S     = x[c] * clamp(p_mask)                         # host-precomputed
    t     = Wp_eff @ S            ; u  = clamp(t)        # MM1
    v     = clamp(u @ p_lin_w.T + p_b)                   # MM2  (p_out)
    z     = Wzp @ v               ; w  = clamp(z)        # MM3
    y     = clamp(w @ z_lin_w.T + z_b)                   # MM4  (z_out)
    out[c] = v + y

Sharding: channels split 8 per NeuronCore (pure data parallel), weights
replicated.  On device the chain alternates between natural and
transposed layouts so that every intermediate is directly usable as the
next matmul's stationary (lhsT) operand -- no transposes anywhere:

    MM1: lhsT=S[k,i]   rhs=Wp_eff.T[k,h]  -> tT[i,h]
    MM2: lhsT=uT[i,h]  rhs=p_lin_w.T[i,j] -> v[h,j]
    MM3: lhsT=v[h,j]   rhs=Wzp.T[h,g]     -> zT[j,g]
    MM4: lhsT=wT[j,g]  rhs=z_lin_w.T[j,m] -> y[g,m]

Everything on device is fp16 (PSUM accumulation fp32): 1 cycle/row PE
throughput with half the DMA traffic of f32, so all four weight
matrices stay SBUF-resident for the whole kernel.

Perf structure (this revision; measured ~813us vs the ~817us of the
plain-pipelined version, PE occupancy >97%):
  * PE warm-up: the Tensor engine's DVFS ramp needs ~6us of continuous
    execution to reach 2.4GHz (it idles at 1.2GHz), and idle gaps over
    ~3us reset it.  A stream of dependency-free dummy matmuls starts
    right after the engine preamble (~6.4us) and bridges the DMA
    prologue (to ~15us), so the ramp completes before the first real
    matmul and channel 0 runs at full clock; sub-2us stalls afterwards
    do not reset the clock.
  * Prologue DMA rides three rings in consumption order: the fast
    software-dynamic scalar ring carries w0's nonzero triangular
    blocks + S slabs 2,3 (and, gated, w1/w2/w3); the slower hardware
    rings (sync/gpsimd, ~65GB/s) carry the other six S slabs.  The
    w1/w2/w3 dma_starts are WAW-gated behind a vector-queue corner
    memset emitted after MM1's nt=0 drains -- the scalar ring shares
    bandwidth round-robin among queued transfers, so queueing 6MB of
    weights early would starve the critical channel-0 chunks.
    spool bufs=1 similarly keeps each next channel's S load (WAR on
    the single S buffer) from entering the sync ring until MM1 of the
    current channel has consumed it.
  * MM1 exploits Wp_eff.T's upper-triangular 128-blocks with k-major
    wide-N instructions: w0 row k is nonzero for all nb >= k, so each
    k contributes one shrinking-width matmul (N=512,384,256,128) --
    36864 of 65536 rows per channel, at wire speed, 96 instructions
    per channel instead of 288 N=128 quarters.
  * Last channel's output cannot hide behind later compute: m<=4
    pieces ride the hardware rings during MM4, the rest rides the
    scalar ring, and the very last tile drains in [P,128] pieces so
    the post-last-matmul flush is short.
"""

import os
import sys

for _p in ("/root/.axon_site/_ro/trn_rl_repo", "/opt/trn_rl_repo"):
    if os.path.isdir(_p) and _p not in sys.path:
        sys.path.append(_p)

import numpy as np

import concourse.bass as bass
import concourse.mybir as mybir
from concourse import bacc
from concourse.tile import TileContext
from concourse.bass_utils import run_bass_kernel_spmd

H = 1024          # hidden == input size
C = 64            # channels
NCORES = 8
CLOC = C // NCORES  # channels per core
P = 128           # SBUF partitions
KO = H // P       # 8 k-blocks
NT = 512          # matmul free-dim tile (1 fp32 PSUM bank)
NN = H // NT      # 2 free-dim tiles

# PE warm-up stream: covers engine-preamble end (~6.4us) through the
# DVFS ramp (~12.4us) and on until channel 0's binding DMA set (S
# slabs 0..3 + w0 lo, ~15.5us) has landed -- starting real matmuls
# earlier just stalls them >3us, which resets the clock ramp.  Big
# dummies run at 427ns (mid-ramp) / 213ns; the N=128 taper keeps
# overrun granularity small.
WARM_BIG = 24     # N=512 dummies
WARM_SMALL = 8    # N=128 dummies

f32 = mybir.dt.float32
f16 = mybir.dt.float16

_cache = {}

# Set by kernel() after each run (for test harness inspection).
last_results = None


def _build(has_pb: bool, has_zb: bool) -> bass.Bass:
    nc = bacc.Bacc(debug=False)

    s = nc.declare_dram_parameter("s", [CLOC, H, H], f16, isOutput=False)
    w_dram = [
        nc.declare_dram_parameter(f"w{i}", [H, H], f16, isOutput=False)
        for i in range(4)
    ]
    pb = zb = None
    if has_pb:
        pb = nc.declare_dram_parameter("pb", [1, H], f16, isOutput=False)
    if has_zb:
        zb = nc.declare_dram_parameter("zb", [1, H], f16, isOutput=False)
    out = nc.declare_dram_parameter("out", [CLOC, H, H], f16, isOutput=True)

    sr = s.ap().rearrange("c (ko p) i -> c p ko i", p=P)
    wr = [w.ap().rearrange("(ko p) n -> p ko n", p=P) for w in w_dram]
    outr = out.ap().rearrange("c (go p) m -> c p go m", p=P)

    with TileContext(nc) as tc:
        with (
            tc.tile_pool(name="const", bufs=1) as constp,
            tc.tile_pool(name="spool", bufs=1) as spool,
            tc.tile_pool(name="uwpool", bufs=1) as uwpool,
            tc.tile_pool(name="w2pool", bufs=1) as w2pool,
            tc.tile_pool(name="vpool", bufs=1) as vpool,
            tc.tile_pool(name="outp", bufs=3) as outp,
            tc.tile_pool(name="psum", bufs=8, space="PSUM") as psum,
        ):
            # ---- persistent weights (loaded once, SBUF-resident) ----
            w0_sb = constp.tile([P, KO, H], f16, tag="w0")
            w1_sb = constp.tile([P, KO, H], f16, tag="w1")
            w2_sb = constp.tile([P, KO, H], f16, tag="w2")
            w3_sb = constp.tile([P, KO, H], f16, tag="w3")
            w_sb = [w0_sb, w1_sb, w2_sb, w3_sb]

            # warm-up matmul operands (memset before use)
            wl_sb = constp.tile([P, P], f16, tag="wl")
            wrm_sb = constp.tile([P, NT], f16, tag="wrm")

            ones_sb = None
            pb_sb = zb_sb = None
            if has_pb or has_zb:
                ones_sb = constp.tile([1, P], f16, tag="ones")
                nc.vector.memset(ones_sb[:], 1.0)
            if has_pb:
                pb_sb = constp.tile([1, H], f16, tag="pb")
            if has_zb:
                zb_sb = constp.tile([1, H], f16, tag="zb")

            # ---- prologue: per-ring DMA in consumption order ----
            # Only sync/gpsimd/scalar can issue DMA.  Measured rings:
            # scalar (software-dynamic, 4KB packets) is the fast one
            # (~200-300GB/s, live ~11.2us); sync (~8.7) and gpsimd
            # (~9.7) are hardware-dynamic rings at ~85GB/s each.  The
            # rings run their queues FIFO, so emission order here IS
            # arrival order (spool bufs=1 keeps the channel-1 S load
            # from entering the sync ring before MM1(c0) finishes).
            # w0's upper-triangular block structure trims the critical
            # w0 traffic to 1.34MB: only (k,nb) 128-blocks with k<=nb
            # are ever read.  Channel-0 nt=0 (S slabs 0..3 + w0 lo)
            # gates the first real matmul ~14.6us -- bridged by the PE
            # warm-up; nt=1 adds slabs 4..7 + w0 hi by ~19us.
            s_cur = spool.tile([P, KO, H], f16, tag="S")

            # gpsimd: warm-up memsets first (gate the dummy matmuls)
            nc.gpsimd.memset(wl_sb[:], 0.0)
            nc.gpsimd.memset(wrm_sb[:], 0.0)

            # sync: s0, s4, s6
            nc.sync.dma_start(s_cur[:, 0, :], sr[0, :, 0, :])
            nc.sync.dma_start(s_cur[:, 4, :], sr[0, :, 4, :])
            nc.sync.dma_start(s_cur[:, 6, :], sr[0, :, 6, :])
            if has_pb:
                nc.sync.dma_start(pb_sb[:], pb.ap())
            if has_zb:
                nc.sync.dma_start(zb_sb[:], zb.ap())
            # gpsimd: s1, s5, s7
            nc.gpsimd.dma_start(s_cur[:, 1, :], sr[0, :, 1, :])
            nc.gpsimd.dma_start(s_cur[:, 5, :], sr[0, :, 5, :])
            nc.gpsimd.dma_start(s_cur[:, 7, :], sr[0, :, 7, :])
            # scalar: w0 nt=0 blocks (k=0 row + k=1:4 staircase), s2,
            # s3, then the w0 hi blocks in nt=1 consumption order
            nc.scalar.dma_start(w0_sb[:, 0:1, :NT], wr[0][:, 0:1, :NT])
            nc.scalar.dma_start(w0_sb[:, 1:4, P:NT], wr[0][:, 1:4, P:NT])
            nc.scalar.dma_start(s_cur[:, 2, :], sr[0, :, 2, :])
            nc.scalar.dma_start(s_cur[:, 3, :], sr[0, :, 3, :])
            nc.scalar.dma_start(w0_sb[:, 0:2, NT:], wr[0][:, 0:2, NT:])
            nc.scalar.dma_start(w0_sb[:, 2:4, NT:], wr[0][:, 2:4, NT:])
            nc.scalar.dma_start(w0_sb[:, 4:6, NT:], wr[0][:, 4:6, NT:])
            nc.scalar.dma_start(w0_sb[:, 6:8, NT + 2 * P:],
                                wr[0][:, 6:8, NT + 2 * P:])
            # w1/w2/w3 are NOT queued here: the scalar ring shares its
            # bandwidth round-robin among queued transfers, so big
            # non-critical loads would starve the critical chunks.
            # Their dma_starts are emitted later, write-after-write
            # gated behind a vector-queue corner memset that runs after
            # MM1 drains (see mm1 mid-hook below).

            # ---- PE warm-up: dependency-free dummy matmuls ----
            # Allocated from the main psum pool so the 8-bank rotation
            # keeps them fully pipelined (a dedicated 1-buf pool chains
            # WAW semaphores and serializes LDWEIGHTS).
            for i in range(WARM_BIG):
                wp = psum.tile([P, NT], f32, tag="ps")
                nc.tensor.matmul(wp[:], wl_sb[:], wrm_sb[:],
                                 start=True, stop=True,
                                 skip_group_check=True)
            for i in range(WARM_SMALL):
                wp = psum.tile([P, NT], f32, tag="ps")
                nc.tensor.matmul(wp[:, :P], wl_sb[:], wrm_sb[:, :P],
                                 start=True, stop=True,
                                 skip_group_check=True)

            def load_s(c):
                st = spool.tile([P, KO, H], f16, tag="S")
                for ko in range(KO):
                    nc.sync.dma_start(st[:, ko, :], sr[c, :, ko, :])
                return st

            def mm_layer(lhsT_sb, rhs_sb, bias_sb, writer):
                # out[m*P:(m+1)*P, nt*NT:(nt+1)*NT] = lhsT.T @ rhs (+bias)
                for m in range(KO):
                    for nt in range(NN):
                        ps = psum.tile([P, NT], f32, tag="ps")
                        for k in range(KO):
                            nc.tensor.matmul(
                                ps[:],
                                lhsT_sb[:, k, m * P:(m + 1) * P],
                                rhs_sb[:, k, nt * NT:(nt + 1) * NT],
                                start=(k == 0),
                                stop=(k == KO - 1 and bias_sb is None),
                            )
                        if bias_sb is not None:
                            # rank-1 accumulate: ones[1,P].T @ bias[1,NT]
                            nc.tensor.matmul(
                                ps[:],
                                ones_sb[:, :],
                                bias_sb[:, nt * NT:(nt + 1) * NT],
                                start=False,
                                stop=True,
                            )
                        writer(m, nt, ps)

            def mm1_layer_kmajor(lhsT_sb, rhs_sb, writer, mid_hook=None):
                # Channel-0 MM1, emitted in k-major order so compute
                # starts as soon as S slab 0 + w0 row 0 land and rides
                # the DMA arrivals (slab k + w0 row k arrive in k
                # order on the scalar ring).  Holds all 8 psum banks
                # per nt phase; drains happen at the last sweep.
                for nt in range(NN):
                    tiles = [psum.tile([P, NT], f32, tag="ps",
                                       name=f"ps_c0_{nt}_{m}")
                             for m in range(KO)]
                    if nt == 0:
                        # quarter staircase, q-sweep: group (m,q) is
                        # k=0..q into quarter q
                        for q in range(4):
                            for m in range(KO):
                                for k in range(q + 1):
                                    nc.tensor.matmul(
                                        tiles[m][:, q * P:(q + 1) * P],
                                        lhsT_sb[:, k, m * P:(m + 1) * P],
                                        rhs_sb[:, k, q * P:(q + 1) * P],
                                        start=(k == 0),
                                        stop=(k == q),
                                    )
                                if q == 3:
                                    writer(m, 0, tiles[m])
                        if mid_hook is not None:
                            mid_hook()
                    else:
                        # k=0..3: N=512 full-bank groups, k-sweep
                        for k in range(4):
                            for m in range(KO):
                                nc.tensor.matmul(
                                    tiles[m][:],
                                    lhsT_sb[:, k, m * P:(m + 1) * P],
                                    rhs_sb[:, k, NT:],
                                    start=(k == 0),
                                    stop=False,
                                    skip_group_check=True,
                                )
                        # k=4..7: quarter staircase on top, k-sweep
                        for k in range(4, KO):
                            for m in range(KO):
                                for q in range(k - 4, 4):
                                    nb = 4 + q
                                    nc.tensor.matmul(
                                        tiles[m][:, q * P:(q + 1) * P],
                                        lhsT_sb[:, k, m * P:(m + 1) * P],
                                        rhs_sb[:, k, nb * P:(nb + 1) * P],
                                        start=False,
                                        stop=(k == nb),
                                        skip_group_check=True,
                                    )
                                if k == KO - 1:
                                    writer(m, 1, tiles[m])

            def mm1_layer(lhsT_sb, rhs_sb, writer, mid_hook=None):
                # MM1's rhs (Wp_eff.T) is upper triangular: 128-block
                # (k, nb) is nonzero only for k <= nb.  nt=0 tiles are a
                # pure quarter staircase (36-of-64 block skip).  nt=1
                # tiles run k=0..3 as one N=512 wire-speed group into
                # the full bank, then finish with a k=4..nb quarter
                # staircase accumulating on top (start=False).
                # nt=0: k-major wide-N -- w0 row k is nonzero for all
                # nb >= k, so one instruction of shrinking width per k
                # (N=512,384,256,128) replaces the 10-instr staircase.
                # PSUM start zeroes the full bank on k=0; stop flags
                # are per-region no-ops on HW (skip_group_check).
                for m in range(KO):
                    ps = psum.tile([P, NT], f32, tag="ps")
                    for k in range(4):
                        nc.tensor.matmul(
                            ps[:, k * P:],
                            lhsT_sb[:, k, m * P:(m + 1) * P],
                            rhs_sb[:, k, k * P:NT],
                            start=(k == 0),
                            stop=(k == 3),
                            skip_group_check=True,
                        )
                    writer(m, 0, ps)
                if mid_hook is not None:
                    mid_hook()
                # nt=1: k=0..3 full-width groups, then the k=4..7
                # staircase rows, again one shrinking-width instruction
                # per k
                for m in range(KO):
                    ps = psum.tile([P, NT], f32, tag="ps")
                    for k in range(4):
                        nc.tensor.matmul(
                            ps[:],
                            lhsT_sb[:, k, m * P:(m + 1) * P],
                            rhs_sb[:, k, NT:],
                            start=(k == 0),
                            stop=False,
                            skip_group_check=True,
                        )
                    for k in range(4, KO):
                        nc.tensor.matmul(
                            ps[:, (k - 4) * P:],
                            lhsT_sb[:, k, m * P:(m + 1) * P],
                            rhs_sb[:, k, NT + (k - 4) * P:],
                            start=False,
                            stop=(k == KO - 1),
                            skip_group_check=True,
                        )
                    writer(m, 1, ps)

            def clamp_into(dst_sb):
                def _w(m, nt, ps):
                    nc.vector.tensor_scalar(
                        dst_sb[:, m, nt * NT:(nt + 1) * NT],
                        ps[:],
                        1.0,
                        -1.0,
                        mybir.AluOpType.min,
                        mybir.AluOpType.max,
                    )
                return _w

            for c in range(CLOC):
                uw = uwpool.tile([P, KO, H], f16, tag="uw")    # uT
                v = vpool.tile([P, KO, H], f16, tag="v")
                wt2 = w2pool.tile([P, KO, H], f16, tag="wt2")  # wT

                if c == 0:
                    def _w1_gate():
                        # vector-queue memset runs after the 8 nt=0
                        # drains; the w1 DMA triggers WAW-wait on it,
                        # keeping the big load off the scalar ring
                        # until the critical chunks have flowed
                        nc.vector.memset(w1_sb[:, 0, :], 0.0)
                        nc.scalar.dma_start(w1_sb[:, :, :NT],
                                            wr[1][:, :, :NT])
                        nc.scalar.dma_start(w1_sb[:, :, NT:],
                                            wr[1][:, :, NT:])
                    mm1_layer(s_cur, w0_sb, clamp_into(uw),
                              mid_hook=_w1_gate)
                    # w2/w3 gated the same way behind the nt=1 drains
                    nc.vector.memset(w2_sb[:, 0, :], 0.0)
                    nc.vector.memset(w3_sb[:, 0, :], 0.0)
                    nc.scalar.dma_start(w2_sb[:, :, :], wr[2][:, :, :])
                    nc.scalar.dma_start(w3_sb[:, :, :], wr[3][:, :, :])
                else:
                    mm1_layer(s_cur, w0_sb, clamp_into(uw))

                # next channel's S loads while this channel computes
                # (spool bufs=2 sequences the buffer reuse)
                if c + 1 < CLOC:
                    s_next = load_s(c + 1)

                mm_layer(uw, w1_sb, pb_sb, clamp_into(v))
                mm_layer(v, w2_sb, None, clamp_into(wt2))

                # ---- MM4 + final out = clamp(ps) + v ----
                last_c = c == CLOC - 1
                ot_holder = [None]

                ring_holder = [0]

                def final_write(m, sl, ps_slice, c=c, v=v,
                                ot_holder=ot_holder, last_c=last_c,
                                eng=None):
                    ot = ot_holder[0]
                    nc.vector.tensor_scalar(
                        ot[:, sl], ps_slice, 1.0, -1.0,
                        mybir.AluOpType.min, mybir.AluOpType.max,
                    )
                    nc.vector.tensor_add(ot[:, sl], ot[:, sl], v[:, m, sl])
                    if eng is not None:
                        eng.dma_start(outr[c, :, m, sl], ot[:, sl])
                    elif last_c and sl.stop == H:
                        # last channel: m=0..5 ride the two hardware
                        # rings (their ~4us transfers hide inside
                        # MM4's 27us); m=6, produced ~3.4us before the
                        # end, and m=7's pieces take the fast scalar
                        # ring so the final flush is short
                        if m == KO - 2:
                            eng2 = nc.scalar
                        else:
                            eng2 = nc.sync if m % 2 == 0 else nc.gpsimd
                        eng2.dma_start(outr[c, :, m, :], ot[:, :])
                    elif sl.stop == H:
                        # stage both halves, then one [P, 1024] DMA:
                        # 2KB descriptor lines instead of 1KB
                        nc.gpsimd.dma_start(outr[c, :, m, :], ot[:, :])

                for m in range(KO):
                    for nt in range(NN):
                        split_tail = last_c and m == KO - 1 and nt == NN - 1
                        ps = psum.tile([P, NT], f32, tag="ps")
                        if nt == 0:
                            ot_holder[0] = outp.tile([P, H], f16, tag="out",
                                                     name="ot")
                        for k in range(KO):
                            nc.tensor.matmul(
                                ps[:],
                                wt2[:, k, m * P:(m + 1) * P],
                                w3_sb[:, k, nt * NT:(nt + 1) * NT],
                                start=(k == 0),
                                stop=(k == KO - 1 and zb_sb is None),
                            )
                        if zb_sb is not None:
                            nc.tensor.matmul(
                                ps[:], ones_sb[:, :],
                                zb_sb[:, nt * NT:(nt + 1) * NT],
                                start=False, stop=True,
                            )
                        if not split_tail:
                            final_write(
                                m, slice(nt * NT, (nt + 1) * NT), ps[:])
                        else:
                            # very last tile: one matmul group, but the
                            # drain+add+DMA goes out in [P,128] pieces
                            # so the post-last-matmul tail is ~1.3us
                            # instead of ~1.7us.  The staged nt=0 half
                            # of m=7 ships first (it only now becomes
                            # safe to overlap with the quarter drains).
                            nc.scalar.dma_start(
                                outr[c, :, m, :NT], ot_holder[0][:, :NT])
                            for q in range(4):
                                col = nt * NT + q * P
                                final_write(
                                    m, slice(col, col + P),
                                    ps[:, q * P:(q + 1) * P],
                                    eng=nc.scalar)

                if c + 1 < CLOC:
                    s_cur = s_next

    nc.compile()  # bacc passes: split multi-waits into event semaphores etc.
    return nc


def _prep_host(x, p_mask, Wp, Wp_diag, Wzp, p_lin_w, p_lin_b, z_lin_w,
               z_lin_b):
    x = np.asarray(x, dtype=np.float32).reshape(C, H, H)
    mask = np.clip(np.asarray(p_mask, dtype=np.float32), -1.0, 1.0)
    s = np.ascontiguousarray((x * mask).astype(np.float16))

    Wp = np.asarray(Wp, dtype=np.float32)
    Wp_eff = np.tril(Wp)
    idx = np.arange(H)
    Wp_eff[idx, idx] = np.clip(np.diagonal(Wp), 0.0, 1.0) + np.asarray(
        Wp_diag, dtype=np.float32
    )
    w = [
        np.ascontiguousarray(Wp_eff.T.astype(np.float16)),
        np.ascontiguousarray(np.asarray(p_lin_w, dtype=np.float32).T.astype(np.float16)),
        np.ascontiguousarray(np.asarray(Wzp, dtype=np.float32).T.astype(np.float16)),
        np.ascontiguousarray(np.asarray(z_lin_w, dtype=np.float32).T.astype(np.float16)),
    ]
    pbh = np.ascontiguousarray(
        np.asarray(p_lin_b, dtype=np.float32).reshape(1, H).astype(np.float16))
    zbh = np.ascontiguousarray(
        np.asarray(z_lin_b, dtype=np.float32).reshape(1, H).astype(np.float16))
    return s, w, pbh, zbh


def kernel(x, p_mask, Wp, Wp_diag, Wzp, p_lin_w, p_lin_b, z_lin_w, z_lin_b):
    global last_results
    s, w, pbh, zbh = _prep_host(
        x, p_mask, Wp, Wp_diag, Wzp, p_lin_w, p_lin_b, z_lin_w, z_lin_b
    )
    has_pb = bool(np.any(pbh))
    has_zb = bool(np.any(zbh))

    key = (has_pb, has_zb)
    if key not in _cache:
        _cache[key] = _build(has_pb, has_zb)
    nc = _cache[key]

    in_maps = []
    for core in range(NCORES):
        m = {
            "s": s[core * CLOC:(core + 1) * CLOC],
            "w0": w[0],
            "w1": w[1],
            "w2": w[2],
            "w3": w[3],
        }
        if has_pb:
            m["pb"] = pbh
        if has_zb:
            m["zb"] = zbh
        in_maps.append(m)

    want_trace = bool(os.environ.get("BASS_TRACE"))
    try:
        res = run_bass_kernel_spmd(
            nc, in_maps, list(range(NCORES)), trace=want_trace
        )
    except ModuleNotFoundError:
        if not want_trace:
            raise
        # profiling hook unavailable in this environment -- run untraced
        res = run_bass_kernel_spmd(
            nc, in_maps, list(range(NCORES)), trace=False
        )
    last_results = res
    out = np.concatenate([r["out"] for r in res.results], axis=0)
    return out.astype(np.float32).reshape(1, C, H, H)
